# revision 113
# baseline (speedup 1.0000x reference)
"""Additive (Bahdanau) attention on 8 trn2 NeuronCores.

reference:
    Q = query @ Wq.T + bq            [B, Lq, d]
    K = key   @ Wk.T + bk            [B, Lk, d]
    scores[b,q,k] = v_w . tanh(Q[b,q,:] + K[b,k,:]) + v_b
    attn = softmax(scores, -1)
    out  = attn @ value
    returns (out, attn)

Sharding: pure data parallel over (b, q-half): core i handles batch i//2,
query rows (i%2)*128 ... +128.  Each core sees the full Lk for its batch, so
softmax is local (no collectives).  v_b shifts all scores equally and cancels
in softmax exactly; no max-subtraction is needed (|score| <= ||v_w||_1 ~ 18,
exp() is safe in fp32).

Algorithm: instead of materializing the B*Lq*Lk*d = 134M-element tanh (which
pins the Scalar engine for ~110us/core), expand tanh in a sine series,

    tanh(s) ~ s/L + sum_r b_r sin(pi r s / L)        (odd, exact Fourier
                                                      coeffs of the ramp-
                                                      corrected periodization;
                                                      L=8.2, R=9 -> sup err
                                                      ~3.5e-3 on |s|<=8.2)

and use sin(w(x+y)) = sin(wx)cos(wy) + cos(wx)sin(wy): every term is now
SEPARABLE in Q and K, so the (q,k) coupling is a plain matmul over d:

    scores = (v.Q)/L + (v.K)/L
           + sum_r [ (b_r v * sin w_r Q) @ cos(w_r K)^T
                   + (b_r v * cos w_r Q) @ sin(w_r K)^T ]

The Sin table spans one period and the DVE ALU has no mod, so arguments are
range-reduced with the magic-number trick: rs = (x*c + 1.5*2^23) - 1.5*2^23
is round-to-nearest(x*c) (two fused tensor_scalars on DVE), the 2-tensor
subtract u = x*c - rs runs on the otherwise-idle GPSIMD, and u in [-.5, .5]
makes Sin(2pi u) = sin(2pi x c) exact.  cos needs no second wrap:
cos(2pi u) = sin(pi/2 - 2pi|u|), with |u| via a sign-bit AND on a uint32
bitcast.  Q and K are concatenated per chunk so each wrap op is one big
instruction; the loop is software-pipelined one harmonic ahead so the
in-order DVE queue never stalls on the GPSIMD/ACT round-trip.  Scores come
out directly in [q, k] layout, so softmax is one Exp with accum_out row
sums.

Harmonic 1 needs no wrap at all (|2 pi c_1 x| < pi fits the table; the
scale rides the ACT affine) and runs split into chunk halves bracketing
the loop: the first half opens the pipeline (fills on a chainless
half-tile) and the second half closes it (shortest possible drain).
The attn half of the result DMAs out as soon as it is scaled, before
the value matmul finishes.

Engine busy (TimelineSim): ACT ~33us (sin/cos passes), DVE ~32us, GPSIMD
~27us, PE ~20us -> ~54us/core vs ~129us for the direct-tanh pipeline, and
more accurate (rel err 2.7e-3 vs 3.1e-3: exact-coefficient series + fp16
features beat the bf16 rounding of the materialized-tanh path).
"""

import sys

for _p in ("/opt/trn_rl_repo",):
    if _p not in sys.path:
        sys.path.insert(0, _p)

import numpy as np

import concourse.bass as bass  # noqa: F401
import concourse.bacc as bacc
import concourse.tile as tile
from concourse import mybir
from concourse import bass_utils
from concourse.bass import ds, ts  # noqa: F401
from concourse.masks import make_identity

F32 = mybir.dt.float32
BF16 = mybir.dt.bfloat16
FP16 = mybir.dt.float16
AF = mybir.ActivationFunctionType

B, LQ, LK, D = 4, 256, 256, 512
NCORES = 8
P = 128          # partitions
QSH = 128        # q rows per core
NCH = D // P     # 4 feature chunks
KT_TILES = LK // P  # 2 k tiles

SER_L = 8.2      # series half-period; data |Q+K| stays well inside
SER_R = 9        # number of harmonics


def _series_coeffs():
    # exact Fourier coefficients of tanh(s) - s/L, odd-periodized on [-L, L]
    L, R = SER_L, SER_R
    s = np.linspace(0, L, 200001)
    G = np.tanh(s) - s / L
    return np.array(
        [2 / L * np.trapezoid(G * np.sin(np.pi * r * s / L), s) for r in range(1, R + 1)],
        np.float64,
    )


def _build():
    nc = bacc.Bacc("TRN2", debug=False, target_bir_lowering=False)
    L, R = SER_L, SER_R

    # pre-chunked on host: [p, c, :] = X[c*128 + p, :]
    qT_d = nc.dram_tensor("qT", [P, NCH, QSH], BF16, kind="ExternalInput").ap()
    kT_d = nc.dram_tensor("kT", [P, NCH, LK], BF16, kind="ExternalInput").ap()
    val_d = nc.dram_tensor("val", [P, KT_TILES, D], FP16, kind="ExternalInput").ap()
    wqT_d = nc.dram_tensor("wqT", [P, NCH, D], BF16, kind="ExternalInput").ap()
    wkT_d = nc.dram_tensor("wkT", [P, NCH, D], BF16, kind="ExternalInput").ap()
    # bias pack: [:, 0:4]=bq, [:, 4:8]=bk (chunked the same way)
    bias_d = nc.dram_tensor("biases", [P, 2 * NCH], F32, kind="ExternalInput").ap()
    # vbr[p, c, r] = v4[p, c] * b_r for r < R;  vbr[p, c, R] = v4[p, c] / L
    vbr_d = nc.dram_tensor("vbr", [P, NCH, SER_R + 1], F32, kind="ExternalInput").ap()
    # single result tensor: [:, :LK] = attn rows, [:, LK:] = out rows
    res_d = nc.dram_tensor("res_o", [QSH, LK + D], F32, kind="ExternalOutput").ap()

    with tile.TileContext(nc) as tc:
        with (
            tc.tile_pool(name="consts", bufs=1) as consts,
            tc.tile_pool(name="weights", bufs=1) as weights,
            tc.tile_pool(name="proj", bufs=1) as proj,
            tc.tile_pool(name="mods", bufs=5) as mods,
            tc.tile_pool(name="feats", bufs=5) as feats,
            tc.tile_pool(name="outs", bufs=1) as outs,
            tc.tile_pool(name="ppsum", bufs=2, space="PSUM") as ppsum,
            tc.tile_pool(name="spsum", bufs=1, space="PSUM") as spsum,
        ):
            # ---------------- constants / ACT table warmup ----------------
            # first ACT op is a Sin -> the trig table set loads at t~0, off
            # the critical path (Identity drains live in the same set).
            warm = consts.tile([P, 1], F32, name="warm", tag="warm")
            nc.vector.memset(warm, 0.0)
            nc.scalar.activation(warm, warm, AF.Sin, scale=2 * np.pi)

            bias_sb = consts.tile([P, 2 * NCH], F32, name="biases_sb", tag="biases_sb")
            nc.sync.dma_start(out=bias_sb, in_=bias_d)
            bq_sb = bias_sb[:, 0:NCH]
            bk_sb = bias_sb[:, NCH : 2 * NCH]
            # K-side first: its projection matmuls overlap the Q-side DMA tail
            wk3 = weights.tile([P, NCH, D], BF16, name="wk3", tag="wk3")
            nc.sync.dma_start(out=wk3, in_=wkT_d)
            kt3 = weights.tile([P, NCH, LK], BF16, name="kt3", tag="kt3")
            nc.sync.dma_start(out=kt3, in_=kT_d)
            wq3 = weights.tile([P, NCH, D], BF16, name="wq3", tag="wq3")
            nc.sync.dma_start(out=wq3, in_=wqT_d)
            qt3 = weights.tile([P, NCH, QSH], BF16, name="qt3", tag="qt3")
            nc.sync.dma_start(out=qt3, in_=qT_d)
            vbr_sb = consts.tile([P, NCH, SER_R + 1], F32, name="vbr_sb", tag="vbr_sb")
            nc.sync.dma_start(out=vbr_sb, in_=vbr_d)

            warm_w = consts.tile([P, P], BF16, name="warm_w", tag="warm_w")
            nc.vector.memset(warm_w, 0.0)
            ident16 = consts.tile([P, P], FP16, name="ident16", tag="ident16")
            make_identity(nc, ident16)
            identf = consts.tile([P, P], F32, name="identf", tag="identf")
            nc.vector.tensor_copy(identf, ident16)
            negidf = consts.tile([P, P], F32, name="negidf", tag="negidf")
            nc.vector.tensor_scalar_mul(negidf, identf, -1.0)
            ones_k = consts.tile([P, LK], FP16, name="ones_k", tag="ones_k")
            nc.vector.memset(ones_k, 1.0)
            ones_q = consts.tile([P, QSH], FP16, name="ones_q", tag="ones_q")
            nc.vector.memset(ones_q, 1.0)
            # value is only needed at the tail; DMA it last.
            val3 = weights.tile([P, KT_TILES, D], FP16, name="val3", tag="val3")
            nc.sync.dma_start(out=val3, in_=val_d)
            val_sb = [val3[:, t_, :] for t_ in range(KT_TILES)]

            # scores accumulate here through the whole harmonic loop
            scores_ps = spsum.tile([P, LK], F32, name="scores_ps", tag="scores_ps")

            # PE HAM warmup: dummy matmuls during the input DMAs so the
            # projection matmuls run at 2.4GHz. Junk is overwritten by the
            # first start=True accumulation.
            for w in range(14):
                nc.tensor.matmul(
                    scores_ps[:, 0:P],
                    warm_w,
                    warm_w,
                    start=True,
                    stop=True,
                    skip_group_check=True,
                )

            # ---------------- projections (fp32 out for the mod wrap) ------
            # Q and K concatenated per chunk: [:, c, 0:QSH]=Q, [:, c, QSH:]=K
            # so every wrap op runs as one big instruction over both sides.
            QK = QSH + LK
            qk_f = proj.tile([P, NCH, QK], F32, name="qk_f", tag="qk_f")
            for c in range(NCH):
                pk = ppsum.tile([P, LK], F32, name=f"pk{c}", tag="proj_ps", bufs=5)
                for dch in range(NCH):
                    nc.tensor.matmul(
                        pk,
                        wk3[:, dch, ts(c, P)],
                        kt3[:, dch, :],
                        start=(dch == 0),
                        stop=(dch == NCH - 1),
                    )
                # drain on ACT (idle in the head); Identity is in every set
                nc.scalar.activation(
                    qk_f[:, c, QSH:], pk, AF.Identity, bias=bk_sb[:, c : c + 1]
                )

                pq = ppsum.tile([P, QSH], F32, name=f"pq{c}", tag="proj_ps", bufs=5)
                for dch in range(NCH):
                    nc.tensor.matmul(
                        pq,
                        wq3[:, dch, ts(c, P)],
                        qt3[:, dch, :],
                        start=(dch == 0),
                        stop=(dch == NCH - 1),
                    )
                nc.scalar.activation(
                    qk_f[:, c, 0:QSH], pq, AF.Identity, bias=bq_sb[:, c : c + 1]
                )

            qk_flat = qk_f.rearrange("p c q -> p (c q)")
            qt_f = qk_f[:, :, 0:QSH]
            kt_f = qk_f[:, :, QSH:]

            # ---------------- ramp term: (v.Q)/L + (v.K)/L ----------------
            qrl = feats.tile([P, NCH, QSH], FP16, name="qrl", tag="qrl")
            krl = feats.tile([P, NCH, LK], FP16, name="krl", tag="krl")
            for c in range(NCH):
                nc.vector.tensor_scalar_mul(
                    qrl[:, c, :], qt_f[:, c, :], vbr_sb[:, c, SER_R : SER_R + 1]
                )
                nc.vector.tensor_scalar_mul(
                    krl[:, c, :], kt_f[:, c, :], vbr_sb[:, c, SER_R : SER_R + 1]
                )
            first_mm = [True]

            def acc(lhsT, rhs, stop=False):
                nc.tensor.matmul(
                    scores_ps, lhsT, rhs, start=first_mm[0], stop=stop
                )
                first_mm[0] = False

            for c in range(NCH):
                acc(qrl[:, c, :], ones_k)       # (v/L . Q)_q broadcast over k
            for c in range(NCH):
                acc(ones_q, krl[:, c, :])       # (v/L . K)_k broadcast over q

            # ---------------- harmonics ----------------
            # No mod on this HW: wrap with magic-number round-to-nearest,
            #   xp = x*c_r;  rs = (xp + M) - M = rn(xp);  u = xp - rs
            # u in [-0.5, 0.5] -> Sin(2pi u) = sin(pi r x / L) exactly.
            # cos needs NO second wrap: cos(2pi u) = sin(pi/2 - 2pi|u|) and
            # |u| <= 0.5 keeps the argument inside the Sin table domain.
            # The per-harmonic weight b_r*v rides the Q-side features
            # (tensor_scalar on GPSIMD, which is otherwise idle).
            MAGIC = float(1.5 * 2**23)
            A = mybir.AluOpType
            pio2 = consts.tile([P, 1], F32, name="pio2", tag="pio2")
            nc.vector.memset(pio2, np.pi / 2)

            NFL = NCH * QK

            def produce(r, c0=0, c1=NCH):
                """DVE arg+round, GPSIMD subtract -> u tile over chunk range
                [c0, c1) (software-pipelined: consumed one iteration later)."""
                cr = r / (2 * L)
                n = (c1 - c0) * QK
                sl = ds(c0 * QK, n)
                xp = mods.tile([P, n], F32, name=f"xp{r}_{c0}", tag="xp")
                nc.vector.tensor_scalar(
                    out=xp, in0=qk_flat[:, sl], scalar1=cr, scalar2=None, op0=A.mult
                )
                rs = mods.tile([P, n], F32, name=f"rs{r}_{c0}", tag="rs")
                nc.vector.tensor_scalar(
                    out=rs, in0=xp, scalar1=MAGIC, scalar2=MAGIC,
                    op0=A.add, op1=A.subtract,
                )
                u = mods.tile([P, n], F32, name=f"u{r}_{c0}", tag="u")
                # halves so downstream abs/sin can start mid-subtract
                h = n // 2
                nc.gpsimd.tensor_sub(u[:, :h], xp[:, :h], rs[:, :h])
                nc.gpsimd.tensor_sub(u[:, h:], xp[:, h:], rs[:, h:])
                return u

            def trig(r, u, c0=0, c1=NCH):
                """abs (DVE) + sin/cos (ACT) for harmonic r, chunks [c0, c1)."""
                nch = c1 - c0
                n = nch * QK
                ua = mods.tile([P, n], F32, name=f"ua{r}_{c0}", tag="ua")
                nc.vector.tensor_scalar(
                    out=ua.bitcast(mybir.dt.uint32),
                    in0=u.bitcast(mybir.dt.uint32),
                    scalar1=0x7FFFFFFF, scalar2=None, op0=A.bitwise_and,
                )
                fsin = feats.tile([P, nch, QK], FP16, name=f"fsin{r}_{c0}", tag="fsin")
                nc.scalar.activation(
                    fsin.rearrange("p c x -> p (c x)"), u, AF.Sin, scale=2 * np.pi
                )
                fcos = feats.tile([P, nch, QK], FP16, name=f"fcos{r}_{c0}", tag="fcos")
                nc.scalar.activation(
                    fcos.rearrange("p c x -> p (c x)"), ua, AF.Sin,
                    bias=pio2, scale=-2 * np.pi,
                )
                return fsin, fcos

            def trig_direct(r, c0, c1):
                """r=1 only: |2 pi c_1 x| <= ~2.2 < pi, so no wrap is needed;
                the c_1 scale rides the ACT affine and only |x| costs a DVE op."""
                cr = r / (2 * L)
                nch = c1 - c0
                n = nch * QK
                sl = ds(c0 * QK, n)
                ua = mods.tile([P, n], F32, name=f"uad{r}_{c0}", tag="ua")
                nc.vector.tensor_scalar(
                    out=ua.bitcast(mybir.dt.uint32),
                    in0=qk_flat[:, sl].bitcast(mybir.dt.uint32),
                    scalar1=0x7FFFFFFF, scalar2=None, op0=A.bitwise_and,
                )
                fsin = feats.tile([P, nch, QK], FP16, name=f"fsind{r}_{c0}", tag="fsin")
                nc.scalar.activation(
                    fsin.rearrange("p c x -> p (c x)"), qk_flat[:, sl], AF.Sin,
                    scale=2 * np.pi * cr,
                )
                fcos = feats.tile([P, nch, QK], FP16, name=f"fcosd{r}_{c0}", tag="fcos")
                nc.scalar.activation(
                    fcos.rearrange("p c x -> p (c x)"), ua, AF.Sin,
                    bias=pio2, scale=-2 * np.pi * cr,
                )
                return fsin, fcos

            def score(r, fsin, fcos, c0=0, c1=NCH, final=False):
                """Q-weighting (DVE) + score matmuls (PE) for harmonic r,
                chunks [c0, c1).  final=True closes the PSUM accumulation."""
                nch = c1 - c0
                uQs = feats.tile([P, nch, QSH], FP16, name=f"uQs{r}_{c0}", tag="uQs")
                uQc = feats.tile([P, nch, QSH], FP16, name=f"uQc{r}_{c0}", tag="uQc")
                for c in range(nch):
                    nc.vector.tensor_scalar_mul(
                        uQs[:, c, :], fsin[:, c, 0:QSH], vbr_sb[:, c0 + c, r - 1 : r]
                    )
                    nc.vector.tensor_scalar_mul(
                        uQc[:, c, :], fcos[:, c, 0:QSH], vbr_sb[:, c0 + c, r - 1 : r]
                    )
                for c in range(nch):
                    acc(uQs[:, c, :], fcos[:, c, QSH:])
                for c in range(nch):
                    acc(uQc[:, c, :], fsin[:, c, QSH:],
                        stop=(final and c == nch - 1))

            # software pipeline: produce(r) runs one iteration ahead of the
            # trig+score consumption; harmonic 1 is split into chunk halves
            # so the cross-engine chain fills on half-sized tiles.
            pend = []
            for r in range(1, R + 1):
                if r == 1:
                    # wrap-free harmonic split in halves: the first half opens
                    # the pipeline (fills on a half-sized, chainless tile);
                    # the second half is deferred to the very end, where its
                    # short chain also shortens the pipeline drain.
                    pend.append((1, None, 0, 2))
                else:
                    pend.append((r, produce(r), 0, NCH))
                while len(pend) > 1:
                    rr, uu, a, b = pend.pop(0)
                    fsin, fcos = (
                        trig_direct(rr, a, b) if uu is None else trig(rr, uu, a, b)
                    )
                    score(rr, fsin, fcos, a, b)
            pend.append((1, None, 2, 4))
            for i, (rr, uu, a, b) in enumerate(pend):
                fsin, fcos = (
                    trig_direct(rr, a, b) if uu is None else trig(rr, uu, a, b)
                )
                score(rr, fsin, fcos, a, b, final=(i == len(pend) - 1))

            # ---------------- softmax + value matmul + outputs ------------
            exp_sb = outs.tile([P, LK], F32, name="exp_sb", tag="exp_sb")
            sums = outs.tile([P, 1], F32, name="sums", tag="sums")
            nc.scalar.activation(exp_sb, scores_ps, AF.Exp, accum_out=sums)
            rsum = outs.tile([P, 1], F32, name="rsum", tag="rsum")
            nc.vector.reciprocal(rsum, sums)

            res_sb = outs.tile([P, LK + D], F32, name="res_sb", tag="res_sb")
            nc.vector.tensor_scalar_mul(res_sb[:, 0:LK], exp_sb, rsum)
            # attn half is ready well before the value matmul: ship it early
            nc.sync.dma_start(out=res_d[:, 0:LK], in_=res_sb[:, 0:LK])

            exp16 = outs.tile([P, LK], FP16, name="exp16", tag="exp16")
            nc.vector.tensor_copy(exp16, exp_sb)
            eT_ps = ppsum.tile([P, LK], FP16, name="eT_ps", tag="proj_ps", bufs=5)
            for t_ in range(KT_TILES):
                nc.tensor.transpose(
                    eT_ps[:, ts(t_, P)], exp16[:, ts(t_, P)], ident16
                )
            eT_sb = outs.tile([P, LK], FP16, name="eT_sb", tag="eT_sb")
            nc.vector.tensor_copy(eT_sb, eT_ps)

            out_ps = ppsum.tile([P, D], F32, name="out_ps", tag="proj_ps", bufs=5)
            for t_ in range(KT_TILES):
                nc.tensor.matmul(
                    out_ps,
                    eT_sb[:, ts(t_, P)],
                    val_sb[t_],
                    start=(t_ == 0),
                    stop=(t_ == KT_TILES - 1),
                )
            nc.vector.tensor_scalar_mul(res_sb[:, LK:], out_ps, rsum)
            nc.sync.dma_start(out=res_d[:, LK:], in_=res_sb[:, LK:])

    nc.compile()
    return nc


_NC_CACHE = None


def _get_nc():
    global _NC_CACHE
    if _NC_CACHE is None:
        _NC_CACHE = _build()
    return _NC_CACHE


def _make_in_maps(query, key, value, Wq, bq, Wk, bk, v_w):
    import ml_dtypes

    f = np.float32
    bf = ml_dtypes.bfloat16

    def chunk(xT, dt):
        # xT: [D, N] -> [P, D//P, N] with [p, c, :] = xT[c*P + p, :]
        xT = np.asarray(xT, f)
        n = xT.shape[1]
        return np.ascontiguousarray(xT.reshape(NCH, P, n).transpose(1, 0, 2).astype(dt))

    wqT = chunk(np.asarray(Wq, f).T, bf)
    wkT = chunk(np.asarray(Wk, f).T, bf)
    biases = np.ascontiguousarray(
        np.concatenate(
            [np.asarray(x, f).reshape(NCH, P).T for x in (bq, bk)], axis=1
        )
    )
    v4 = np.asarray(v_w, np.float64).reshape(NCH, P).T  # [P, NCH]
    coef = np.concatenate([_series_coeffs(), [1.0 / SER_L]])  # [R+1]
    vbr = np.ascontiguousarray(
        (v4[:, :, None] * coef[None, None, :]).astype(f)
    )
    query = np.asarray(query, f)
    key = np.asarray(key, f)
    value = np.asarray(value, f)

    in_maps = []
    for i in range(NCORES):
        b, half = divmod(i, 2)
        q0 = half * QSH
        valb = value[b].reshape(KT_TILES, P, D).transpose(1, 0, 2).astype(np.float16)
        in_maps.append(
            dict(
                qT=chunk(query[b, q0 : q0 + QSH, :].T, bf),
                kT=chunk(key[b].T, bf),
                val=np.ascontiguousarray(valb),
                wqT=wqT,
                wkT=wkT,
                biases=biases,
                vbr=vbr,
            )
        )
    return in_maps


def run_spmd(query, key, value, Wq, bq, Wk, bk, v_w, **run_kwargs):
    """Run on the 8 cores; returns (out, attn, BassKernelResults)."""
    nc = _get_nc()
    in_maps = _make_in_maps(query, key, value, Wq, bq, Wk, bk, v_w)
    res = bass_utils.run_bass_kernel_spmd(
        nc, in_maps, core_ids=list(range(NCORES)), **run_kwargs
    )
    out = np.empty((B, LQ, D), np.float32)
    attn = np.empty((B, LQ, LK), np.float32)
    for i in range(NCORES):
        b, half = divmod(i, 2)
        q0 = half * QSH
        r = res.results[i]["res_o"]
        attn[b, q0 : q0 + QSH, :] = r[:, :LK]
        out[b, q0 : q0 + QSH, :] = r[:, LK:]
    return out, attn, res


def kernel(query, key, value, Wq, bq, Wk, bk, v_w, v_b):
    # v_b shifts every score equally -> cancels in softmax; unused.
    out, attn, _ = run_spmd(query, key, value, Wq, bq, Wk, bk, v_w)
    return out, attn
